# revision 41
# baseline (speedup 1.0000x reference)
"""ConceptCLIP loss kernel for 8x Trainium2 NeuronCores (Bass/Tile).

Strategy (data-parallel over the image batch axis m):
  - Each core owns 16 of the 128 images; concept/text features are replicated.
  - Concepts are host-packed (only w < counts[v] kept) and quantized to
    fp8e4m3 RAW; their 1/||c|| is folded into the G gather matrix on device.
  - Patches are host-TRANSPOSED to (d, n) layout (pure layout, free) and
    shipped bf16. Per-patch sum-of-squares: fp8 squares (ACT) + a
    (1/256)-vector fp8-DoubleRow matmul on PE giving (||x||/16)^2 in row
    form; sqrt -> reciprocal_approx_fast -> GPSIMD partition_broadcast ->
    multiply (DVE early pairs / GPSIMD late pairs) yields fp8 operands
    holding 16*x/||x|| (single quantization).
  - Main loop: fp8 DoubleRow matmuls (contraction 256/instr) in 2 phases of
    4 image pairs; per (c, phase) four single-bank [128,512] PSUM tiles from
    a 6-deep ring; per-pair DVE reduce_max drains (392 valid cols).
  - S = (G*rnorm/16)^T @ maxcol in bf16; IT-align logits via raw bf16 matmul
    with deferred rank-1 normalization. Softplus loss elements are DMA'd out;
    host sums them.
"""

import math
import os
import sys

for _p in ("/opt/trn_rl_repo", "/root/.axon_site/_ro/trn_rl_repo"):
    if os.path.isdir(_p) and _p not in sys.path:
        sys.path.insert(0, _p)

import ml_dtypes
import numpy as np

import concourse.tile as tile
from concourse import bacc, mybir
from concourse.bass_utils import run_bass_kernel_spmd

BF16 = ml_dtypes.bfloat16
FP8 = ml_dtypes.float8_e4m3

N_CORES = 8
B, NPATCH, D, W = 128, 196, 768, 32
M_PER = B // N_CORES   # 16 images per core
PAIRS = M_PER // 2     # 8 image pairs
KC = D // 128          # 6 contraction chunks of 128
NKP = KC // 2          # 3 DoubleRow k-pairs (contraction 256 each)
VCOLS = 2 * NPATCH     # 392 valid columns per pair (imgA 0:196, imgB 196:392)
COLS = 400             # padded to 16B-aligned k-chunk stride

F32 = mybir.dt.float32
BF = mybir.dt.bfloat16
F8 = mybir.dt.float8e4
AX = mybir.AxisListType
AF = mybir.ActivationFunctionType
DR = mybir.MatmulPerfMode.DoubleRow

_cache = {}


def _build(C, t, bias):
    """Build + compile the per-core Bass program. C = number of 128-row packed
    concept chunks; t/bias are compile-time scalar constants."""
    P2 = C * 128
    nc = bacc.Bacc("TRN2", target_bir_lowering=False, debug=False,
                   num_devices=N_CORES)

    d_pT = nc.dram_tensor("pT", (PAIRS, 128, KC, COLS), BF, kind="ExternalInput")
    d_cT = nc.dram_tensor("cT", (NKP, 128, 2, P2), F8, kind="ExternalInput")
    d_cnat = nc.dram_tensor("cnat", (P2, D), BF, kind="ExternalInput")
    d_GT = nc.dram_tensor("GT", (C, 128, B), BF, kind="ExternalInput")
    d_txtT = nc.dram_tensor("txtT", (128, KC, 128), BF, kind="ExternalInput")
    d_imgT = nc.dram_tensor("imgT", (128, KC, M_PER), BF, kind="ExternalInput")
    d_txtn = nc.dram_tensor("txtn", (B, D), BF, kind="ExternalInput")
    d_sign = nc.dram_tensor("signneg", (B, M_PER), F32, kind="ExternalInput")
    d_rc = nc.dram_tensor("rc_el", (B, M_PER), F32, kind="ExternalOutput")
    d_it = nc.dram_tensor("it_el", (B, M_PER), F32, kind="ExternalOutput")

    with tile.TileContext(nc) as tc:
        with (
            tc.tile_pool(name="consts", bufs=1) as consts,
            tc.tile_pool(name="work", bufs=3) as work,
            tc.tile_pool(name="small", bufs=4) as small,
            tc.tile_pool(name="psum", bufs=6, space="PSUM") as psum,
        ):
            pT = [consts.tile([128, KC, COLS], BF, tag=f"pT{p}", name=f"pT{p}")
                  for p in range(PAIRS)]
            for p in range(4):
                nc.sync.dma_start(out=pT[p][:], in_=d_pT.ap()[p])
            sign = consts.tile([B, M_PER], F32, tag="sign")
            nc.sync.dma_start(out=sign[:], in_=d_sign.ap())
            ones_col = consts.tile([128, 1], BF, tag="ones")
            nc.vector.memset(ones_col[:], 1.0)
            sc8 = consts.tile([128, 2, 16], F8, tag="sc")
            nc.vector.memset(sc8[:], 1.0 / 256.0)
            warm = small.tile([1, 1], F32, tag="warm")
            nc.vector.memset(warm[:], 1.0)
            nc.scalar.activation(out=warm[:], in_=warm[:], func=AF.Square)

            maxcol = consts.tile([128, C, M_PER], BF, tag="maxcol")
            css = consts.tile([128, C], F32, tag="css")
            csq = consts.tile([128, C], F32, tag="csq")
            crn = consts.tile([128, C], F32, tag="crn")
            GTbf = consts.tile([128, C * B], BF, tag="GT")
            for c in range(C):
                nc.sync.dma_start(out=GTbf[:, c * B:(c + 1) * B], in_=d_GT.ap()[c])
            yit = consts.tile([B, M_PER], F32, tag="yit")

            rhs8 = [consts.tile([128, KC, COLS], F8, tag=f"r8{p}", name=f"r8{p}")
                    for p in range(PAIRS)]
            cT = []
            for j in range(NKP):
                tj = consts.tile([128, 2, P2], F8, tag=f"cT{j}", name=f"cT{j}")
                nc.sync.dma_start(out=tj[:], in_=d_cT.ap()[j])
                cT.append(tj)

            sq_tiles = {}

            def prep_squares(pr, fast=False):
                # sq = (pT)^2 in fp8 (ACT; GPSIMD helps for the early pairs)
                if pr >= 4:
                    nc.sync.dma_start(out=pT[pr][:], in_=d_pT.ap()[pr])
                sq = work.tile([128, KC, COLS], F8, tag="sq", bufs=5)
                for k in range(KC):
                    if fast and k >= 3:
                        nc.gpsimd.tensor_mul(sq[:, k, :], pT[pr][:, k, :],
                                             pT[pr][:, k, :])
                    else:
                        nc.scalar.activation(out=sq[:, k, :], in_=pT[pr][:, k, :],
                                             func=AF.Square)
                sq_tiles[pr] = sq

            def prep_finish(pr, defer_mult=False):
                # (1/256)-DR-matmul -> (||x||/16)^2 row; sqrt; fast recip;
                # broadcast; scale pT to fp8 (16*x/||x||).
                bt = psum.tile([128, 512], F32, tag="aux", bufs=2, name="bt")
                for j in range(NKP):
                    nc.tensor.matmul(bt[0:1, 0:COLS], lhsT=sc8[:, :, 0:1],
                                     rhs=sq_tiles[pr][:, 2 * j:2 * j + 2, :],
                                     start=(j == 0), stop=(j == NKP - 1),
                                     perf_mode=DR)
                del sq_tiles[pr]
                srow = small.tile([1, COLS], F32, tag="srow", bufs=2)
                nc.scalar.sqrt(srow[:], bt[0:1, 0:COLS])
                rrec = small.tile([1, COLS], F32, tag="rrec", bufs=2)
                nc.vector.reciprocal_approx_fast(rrec[:], srow[:])  # 16/||x||
                bc = work.tile([128, COLS], F32, tag="bc", bufs=5)
                nc.gpsimd.partition_broadcast(out_ap=bc[:, :], in_ap=rrec[0:1, :])
                if defer_mult:
                    return bc
                for k in range(KC):
                    nc.vector.tensor_mul(rhs8[pr][:, k, :], pT[pr][:, k, :],
                                         bc[:, :])

            def cnat_step(cc):
                cn = work.tile([128, D], BF, tag="cnat", bufs=3)
                nc.sync.dma_start(out=cn[:], in_=d_cnat.ap()[cc * 128:(cc + 1) * 128, :])
                scr = work.tile([128, D], BF, tag="scr", bufs=3)
                nc.scalar.activation(out=scr[:], in_=cn[:], func=AF.Square,
                                     accum_out=css[:, cc:cc + 1])
                nc.scalar.sqrt(csq[:, cc:cc + 1], css[:, cc:cc + 1])
                nc.vector.reciprocal_approx_fast(crn[:, cc:cc + 1],
                                                 csq[:, cc:cc + 1])

            for pr in range(4):
                prep_squares(pr, fast=True)
            bcs = [prep_finish(pr, defer_mult=True) for pr in range(4)]
            # k-outer so phase A can start after the first k-chunks land
            for k in range(KC):
                for pr in range(4):
                    nc.vector.tensor_mul(rhs8[pr][:, k, :], pT[pr][:, k, :],
                                         bcs[pr][:, :])

            def main_phase(half, preps=()):
                # 4 pairs; per (c, pair) one single-bank [128,512] PSUM tile
                # from a 6-deep ring; per-pair reduce_max drain (392 cols).
                preps = dict(preps)
                prs = list(range(half * 4, half * 4 + 4))
                for c in range(C):
                    pss = [psum.tile([128, 2, 512], F32, tag="mm", bufs=3,
                                     name="mm") for _ in range(2)]
                    for j in range(NKP):
                        for i, pr in enumerate(prs):
                            nc.tensor.matmul(pss[i // 2][:, i % 2, 0:COLS],
                                             lhsT=cT[j][:, :, c * 128:(c + 1) * 128],
                                             rhs=rhs8[pr][:, 2 * j:2 * j + 2, :],
                                             start=(j == 0), stop=(j == NKP - 1),
                                             perf_mode=DR)
                    for g in range(2):
                        p0 = prs[2 * g]
                        nc.vector.reduce_max(
                            out=maxcol[:, c, 2 * p0:2 * p0 + 4].rearrange(
                                "p (b s) -> p b s", s=2),
                            in_=pss[g][:, :, 0:VCOLS].rearrange(
                                "p b (s x) -> p b s x", s=2),
                            axis=AX.X)
                    for fn in preps.get(c, ()):
                        fn()

            # ---- IT-align: raw bf16 matmul + deferred rank-1 normalization --
            txtT = consts.tile([128, KC, 128], BF, tag="txtT")
            nc.sync.dma_start(out=txtT[:], in_=d_txtT.ap())
            imgT = consts.tile([128, KC, M_PER], BF, tag="imgT")
            nc.sync.dma_start(out=imgT[:], in_=d_imgT.ap())
            txtn = work.tile([128, D], BF, tag="cnat", bufs=3)
            nc.sync.dma_start(out=txtn[:], in_=d_txtn.ap())
            tscr = work.tile([128, D], BF, tag="scr", bufs=3)
            tss = small.tile([128, 1], F32, tag="tss")
            nc.scalar.activation(out=tscr[:], in_=txtn[:], func=AF.Square,
                                 accum_out=tss[:])
            nc.scalar.sqrt(tss[:], tss[:])
            av = small.tile([128, 1], F32, tag="av")
            nc.vector.reciprocal(av[:], tss[:])
            nc.vector.tensor_scalar_mul(av[:], av[:], float(t))  # t/||txt_v||

            # img norms via ones-matmul on squared imgT (transposed layout)
            isq = small.tile([128, KC, M_PER], BF, tag="isq")
            nc.scalar.activation(out=isq[:], in_=imgT[:], func=AF.Square)
            aux = psum.tile([128, 512], F32, tag="aux", bufs=2, name="aux")
            for k in range(KC):
                nc.tensor.matmul(aux[0:1, 0:M_PER], lhsT=ones_col[:, :],
                                 rhs=isq[:, k, :], start=(k == 0),
                                 stop=(k == KC - 1))
            ib = small.tile([1, M_PER], F32, tag="ib")
            nc.scalar.sqrt(ib[:], aux[0:1, 0:M_PER])
            nc.vector.reciprocal(ib[:], ib[:])                   # 1/||img_m||
            bg = small.tile([128, M_PER], F32, tag="bg")
            nc.gpsimd.partition_broadcast(out_ap=bg[:, :], in_ap=ib[0:1, :])

            itp = psum.tile([128, 512], F32, tag="aux", bufs=2, name="itp")
            for k in range(KC):
                nc.tensor.matmul(itp[:, 0:M_PER], lhsT=txtT[:, k, :],
                                 rhs=imgT[:, k, :], start=(k == 0),
                                 stop=(k == KC - 1))
            nc.scalar.activation(out=yit[:], in_=itp[:, 0:M_PER], func=AF.Copy,
                                 scale=av[:])
            nc.vector.tensor_mul(yit[:], yit[:], bg[:])
            nc.vector.tensor_scalar_add(yit[:], yit[:], float(bias))

            nc.scalar.activation(out=warm[:], in_=warm[:], func=AF.Exp)

            main_phase(0, preps={
                0: (lambda: prep_squares(4),),
                1: (lambda: prep_squares(5),),
                2: (lambda: prep_finish(4), lambda: prep_squares(6)),
                3: (lambda: prep_squares(7),),
                4: (lambda: prep_finish(5),),
                6: (lambda: prep_finish(6),),
                8: (lambda: prep_finish(7),),
            })

            # ---- S[v, m] = sum_p G[p,v] * maxcol[p,m]/||c_p||  ------------
            # (fp32 acc; accumulated chunk-by-chunk as phase B drains land)
            sps = psum.tile([128, 512], F32, tag="aux", bufs=2, name="sps")

            def s_step(cc):
                mcs = small.tile([128, M_PER], BF, tag="mcs", bufs=3)
                nc.vector.tensor_scalar_mul(mcs[:], maxcol[:, cc, :],
                                             crn[:, cc:cc + 1])
                nc.tensor.matmul(sps[:, 0:M_PER], lhsT=GTbf[:, cc * B:(cc + 1) * B],
                                 rhs=mcs[:], start=(cc == 0), stop=(cc == C - 1))

            def phase_b_prep(cc):
                fns = [lambda cc=cc: cnat_step(cc)]
                if cc >= 2:
                    fns.append(lambda cc=cc: s_step(cc - 2))
                return tuple(fns)

            main_phase(1, preps={cc: phase_b_prep(cc) for cc in range(C)})
            s_step(C - 2)
            s_step(C - 1)

            def softplus_out(y_ap, d_out):
                el = small.tile([B, M_PER], F32, tag="el", name="el")
                nc.scalar.activation(out=el[:], in_=y_ap, func=AF.Exp)
                nc.vector.tensor_scalar_add(el[:], el[:], 1.0)
                nc.scalar.activation(out=el[:], in_=el[:], func=AF.Ln)
                nc.sync.dma_start(out=d_out.ap(), in_=el[:])

            yrc = small.tile([B, M_PER], F32, tag="y")
            nc.scalar.activation(out=yrc[:], in_=sps[:, 0:M_PER], func=AF.Copy,
                                 bias=float(bias), scale=float(t))
            nc.vector.tensor_mul(yrc[:], yrc[:], sign[:])
            softplus_out(yrc[:], d_rc)

            nc.vector.tensor_mul(yit[:], yit[:], sign[:])
            softplus_out(yit[:], d_it)

    nc.compile()
    return nc


def _install_trace_hook():
    """Register the axon NTFF profiling hook (missing from this image) so
    run_bass_kernel_spmd(trace=True) can capture HW exec time."""
    import contextlib
    import ctypes
    import types

    import concourse.bass_utils as bu

    if "antenv.axon_hooks" in sys.modules:
        return
    so_path = "/opt/axon/libaxon_pjrt.so"

    def _make_hook():
        lib = ctypes.CDLL(so_path)
        if not hasattr(lib, "axon_start_nrt_profile"):
            return None
        lib.axon_start_nrt_profile.argtypes = [ctypes.POINTER(ctypes.c_int64),
                                               ctypes.c_size_t]
        lib.axon_start_nrt_profile.restype = ctypes.c_int64
        lib.axon_stop_nrt_profile.argtypes = [ctypes.c_char_p]
        lib.axon_stop_nrt_profile.restype = ctypes.c_int64

        @contextlib.contextmanager
        def _hook(output_dir, device_ids):
            import jax
            jax.devices()
            if device_ids:
                ids = (ctypes.c_int64 * len(device_ids))(*device_ids)
                rc = lib.axon_start_nrt_profile(ids, len(device_ids))
            else:
                rc = lib.axon_start_nrt_profile(None, 0)
            if rc != 0:
                raise RuntimeError(f"axon_start_nrt_profile rc={rc}")
            try:
                yield
            finally:
                n = lib.axon_stop_nrt_profile(str(output_dir).encode())
                print(f"profile: {n} file(s) written to {output_dir}",
                      file=sys.stderr)

        return _hook

    mod = types.ModuleType("antenv.axon_hooks")
    mod.get_axon_ntff_profile_hook = _make_hook
    sys.modules["antenv.axon_hooks"] = mod
    bu.upload_artifacts = lambda tmpdir: tmpdir  # no S3 in this container


def _prepare(inputs):
    image_features = np.asarray(inputs["image_features"], np.float32)
    text_features = np.asarray(inputs["text_features"], np.float32)
    image_token_features = np.asarray(inputs["image_token_features"], np.float32)
    concept_text_features = np.asarray(inputs["concept_text_features"], np.float32)
    counts = np.asarray(inputs["concept_counts"]).astype(np.int64)
    t = float(np.exp(np.clip(np.float32(inputs["logit_scale"]), -10.0, 10.0)))
    bias = float(np.float32(inputs["logit_bias"]))

    # pack concepts: keep only w < counts[v]; pad rows with ones (zero weight)
    vidx = np.repeat(np.arange(B), counts)
    widx = np.concatenate([np.arange(c) for c in counts])
    P = len(vidx)
    C = math.ceil(P / 128)
    P2 = C * 128
    cnat = np.ones((P2, D), np.float32)
    cnat[:P] = concept_text_features[vidx, widx]
    cnat_bf = cnat.astype(BF16)
    # cT[j, d128, h, p] = fp8(cnat[p, (2j+h)*128 + d])
    cT = np.ascontiguousarray(
        cnat.astype(FP8).T.reshape(NKP, 2, 128, P2).transpose(0, 2, 1, 3))

    # G with 1/(16*counts): folds away the x16 patch scale
    G = np.zeros((P2, B), np.float32)
    G[np.arange(P), vidx] = 1.0 / (16.0 * counts[vidx])
    GT = G.astype(BF16).reshape(C, 128, B)

    txt_bf = text_features.astype(BF16)
    # txtT[d, k, v] = txt_bf[v, k*128 + d]
    txtT = np.ascontiguousarray(
        txt_bf.T.reshape(KC, 128, B).transpose(1, 0, 2))

    in_maps = []
    for core in range(N_CORES):
        s = slice(core * M_PER, (core + 1) * M_PER)
        sh = image_token_features[s].astype(BF16)        # (16, 196, 768)
        pT = np.zeros((PAIRS, 128, KC, COLS), BF16)
        shT = sh.transpose(0, 2, 1).reshape(M_PER, KC, 128, NPATCH)
        # pT[pr, d, k, 196*i2 + n] = patches[2pr+i2][n, k*128+d]
        pT[:, :, :, 0:NPATCH] = shT[0::2].transpose(0, 2, 1, 3)
        pT[:, :, :, NPATCH:VCOLS] = shT[1::2].transpose(0, 2, 1, 3)
        img_bf = image_features[s].astype(BF16)          # (16, 768)
        imgT = np.ascontiguousarray(
            img_bf.T.reshape(KC, 128, M_PER).transpose(1, 0, 2))
        signneg = np.ones((B, M_PER), np.float32)
        for j in range(M_PER):
            signneg[core * M_PER + j, j] = -1.0
        in_maps.append({
            "pT": pT,
            "cT": cT,
            "cnat": cnat_bf,
            "GT": GT,
            "txtT": txtT,
            "imgT": imgT,
            "txtn": txt_bf,
            "signneg": signneg,
        })
    return in_maps, C, t, bias


def _run(inputs, trace=False, tmpdir=None):
    in_maps, C, t, bias = _prepare(inputs)
    key = (C, t, bias)
    if key not in _cache:
        _cache[key] = _build(C, t, bias)
    nc = _cache[key]
    kwargs = {}
    if trace:
        _install_trace_hook()
        kwargs = dict(trace=True, tmpdir=tmpdir)
    res = run_bass_kernel_spmd(nc, in_maps, core_ids=list(range(N_CORES)),
                               **kwargs)
    it_sum = sum(float(r["it_el"].astype(np.float64).sum()) for r in res.results)
    rc_sum = sum(float(r["rc_el"].astype(np.float64).sum()) for r in res.results)
    it_loss = it_sum / (B * B)
    rc_loss = rc_sum / (B * B)
    total = it_loss + 0.5 * rc_loss
    out = (np.float32(total), np.float32(it_loss), np.float32(rc_loss))
    return out, res


def kernel(**inputs):
    out, _ = _run(inputs)
    return out


# revision 42
# speedup vs baseline: 1.0138x; 1.0138x over previous
"""ConceptCLIP loss kernel for 8x Trainium2 NeuronCores (Bass/Tile).

Strategy (data-parallel over the image batch axis m):
  - Each core owns 16 of the 128 images; concept/text features are replicated.
  - Concepts are host-packed (only w < counts[v] kept) and quantized to
    fp8e4m3 RAW; their 1/||c|| is folded into the G gather matrix on device.
  - Patches are host-TRANSPOSED to (d, n) layout (pure layout, free) and
    shipped bf16. Per-patch sum-of-squares: fp8 squares (ACT) + a
    (1/256)-vector fp8-DoubleRow matmul on PE giving (||x||/16)^2 in row
    form; sqrt -> reciprocal_approx_fast -> GPSIMD partition_broadcast ->
    multiply (DVE early pairs / GPSIMD late pairs) yields fp8 operands
    holding 16*x/||x|| (single quantization).
  - Main loop: fp8 DoubleRow matmuls (contraction 256/instr) in 2 phases of
    4 image pairs; per (c, phase) four single-bank [128,512] PSUM tiles from
    a 6-deep ring; per-pair DVE reduce_max drains (392 valid cols).
  - S = (G*rnorm/16)^T @ maxcol in bf16; IT-align logits via raw bf16 matmul
    with deferred rank-1 normalization. Softplus loss elements are DMA'd out;
    host sums them.
"""

import math
import os
import sys

for _p in ("/opt/trn_rl_repo", "/root/.axon_site/_ro/trn_rl_repo"):
    if os.path.isdir(_p) and _p not in sys.path:
        sys.path.insert(0, _p)

import ml_dtypes
import numpy as np

import concourse.tile as tile
from concourse import bacc, mybir
from concourse.bass_utils import run_bass_kernel_spmd

BF16 = ml_dtypes.bfloat16
FP8 = ml_dtypes.float8_e4m3

N_CORES = 8
B, NPATCH, D, W = 128, 196, 768, 32
M_PER = B // N_CORES   # 16 images per core
PAIRS = M_PER // 2     # 8 image pairs
KC = D // 128          # 6 contraction chunks of 128
NKP = KC // 2          # 3 DoubleRow k-pairs (contraction 256 each)
VCOLS = 2 * NPATCH     # 392 valid columns per pair (imgA 0:196, imgB 196:392)
COLS = 400             # padded to 16B-aligned k-chunk stride

F32 = mybir.dt.float32
BF = mybir.dt.bfloat16
F8 = mybir.dt.float8e4
AX = mybir.AxisListType
AF = mybir.ActivationFunctionType
DR = mybir.MatmulPerfMode.DoubleRow

_cache = {}


def _build(C, t, bias):
    """Build + compile the per-core Bass program. C = number of 128-row packed
    concept chunks; t/bias are compile-time scalar constants."""
    P2 = C * 128
    nc = bacc.Bacc("TRN2", target_bir_lowering=False, debug=False,
                   num_devices=N_CORES)

    d_pT = nc.dram_tensor("pT", (PAIRS, 128, KC, COLS), BF, kind="ExternalInput")
    d_cT = nc.dram_tensor("cT", (NKP, 128, 2, P2), F8, kind="ExternalInput")
    d_cnat = nc.dram_tensor("cnat", (P2, D), BF, kind="ExternalInput")
    d_GT = nc.dram_tensor("GT", (C, 128, B), BF, kind="ExternalInput")
    d_txtT = nc.dram_tensor("txtT", (128, KC, 128), BF, kind="ExternalInput")
    d_imgT = nc.dram_tensor("imgT", (128, KC, M_PER), BF, kind="ExternalInput")
    d_txtn = nc.dram_tensor("txtn", (B, D), BF, kind="ExternalInput")
    d_sign = nc.dram_tensor("signneg", (B, M_PER), F32, kind="ExternalInput")
    d_rc = nc.dram_tensor("rc_el", (B, M_PER), F32, kind="ExternalOutput")
    d_it = nc.dram_tensor("it_el", (B, M_PER), F32, kind="ExternalOutput")

    with tile.TileContext(nc) as tc:
        with (
            tc.tile_pool(name="consts", bufs=1) as consts,
            tc.tile_pool(name="work", bufs=3) as work,
            tc.tile_pool(name="small", bufs=4) as small,
            tc.tile_pool(name="psum", bufs=6, space="PSUM") as psum,
        ):
            pT = [consts.tile([128, KC, COLS], BF, tag=f"pT{p}", name=f"pT{p}")
                  for p in range(PAIRS)]
            for p in range(4):
                nc.sync.dma_start(out=pT[p][:], in_=d_pT.ap()[p])
            sign = consts.tile([B, M_PER], F32, tag="sign")
            nc.sync.dma_start(out=sign[:], in_=d_sign.ap())
            ones_col = consts.tile([128, 1], BF, tag="ones")
            nc.vector.memset(ones_col[:], 1.0)
            sc8 = consts.tile([128, 2, 16], F8, tag="sc")
            nc.vector.memset(sc8[:], 1.0 / 256.0)
            warm = small.tile([1, 1], F32, tag="warm")
            nc.vector.memset(warm[:], 1.0)
            nc.scalar.activation(out=warm[:], in_=warm[:], func=AF.Square)

            maxcol = consts.tile([128, C, M_PER], BF, tag="maxcol")
            css = consts.tile([128, C], F32, tag="css")
            csq = consts.tile([128, C], F32, tag="csq")
            crn = consts.tile([128, C], F32, tag="crn")
            GTbf = consts.tile([128, C * B], BF, tag="GT")
            for c in range(C):
                nc.sync.dma_start(out=GTbf[:, c * B:(c + 1) * B], in_=d_GT.ap()[c])
            yit = consts.tile([B, M_PER], F32, tag="yit")

            rhs8 = [consts.tile([128, KC, COLS], F8, tag=f"r8{p}", name=f"r8{p}")
                    for p in range(PAIRS)]
            cT = []
            for j in range(NKP):
                tj = consts.tile([128, 2, P2], F8, tag=f"cT{j}", name=f"cT{j}")
                nc.sync.dma_start(out=tj[:], in_=d_cT.ap()[j])
                cT.append(tj)

            sq_tiles = {}

            def prep_squares(pr, fast=False):
                # sq = (pT)^2 in fp8 (ACT; GPSIMD helps for the early pairs)
                if pr >= 4:
                    nc.sync.dma_start(out=pT[pr][:], in_=d_pT.ap()[pr])
                sq = work.tile([128, KC, COLS], F8, tag="sq", bufs=5)
                for k in range(KC):
                    if fast and k >= 3:
                        nc.gpsimd.tensor_mul(sq[:, k, :], pT[pr][:, k, :],
                                             pT[pr][:, k, :])
                    else:
                        nc.scalar.activation(out=sq[:, k, :], in_=pT[pr][:, k, :],
                                             func=AF.Square)
                sq_tiles[pr] = sq

            def prep_finish(pr, defer_mult=False):
                # (1/256)-DR-matmul -> (||x||/16)^2 row; sqrt; fast recip;
                # broadcast; scale pT to fp8 (16*x/||x||).
                bt = psum.tile([128, 512], F32, tag="aux", bufs=2, name="bt")
                for j in range(NKP):
                    nc.tensor.matmul(bt[0:1, 0:COLS], lhsT=sc8[:, :, 0:1],
                                     rhs=sq_tiles[pr][:, 2 * j:2 * j + 2, :],
                                     start=(j == 0), stop=(j == NKP - 1),
                                     perf_mode=DR)
                del sq_tiles[pr]
                srow = small.tile([1, COLS], F32, tag="srow", bufs=2)
                nc.scalar.sqrt(srow[:], bt[0:1, 0:COLS])
                rrec = small.tile([1, COLS], F32, tag="rrec", bufs=2)
                nc.vector.reciprocal_approx_fast(rrec[:], srow[:])  # 16/||x||
                bc = work.tile([128, COLS], F32, tag="bc", bufs=5)
                nc.gpsimd.partition_broadcast(out_ap=bc[:, :], in_ap=rrec[0:1, :])
                if defer_mult:
                    return bc
                for k in range(KC):
                    nc.vector.tensor_mul(rhs8[pr][:, k, :], pT[pr][:, k, :],
                                         bc[:, :])

            def cnat_step(cc):
                cn = work.tile([128, D], BF, tag="cnat", bufs=3)
                nc.sync.dma_start(out=cn[:], in_=d_cnat.ap()[cc * 128:(cc + 1) * 128, :])
                scr = work.tile([128, D], BF, tag="scr", bufs=3)
                nc.scalar.activation(out=scr[:], in_=cn[:], func=AF.Square,
                                     accum_out=css[:, cc:cc + 1])
                nc.scalar.sqrt(csq[:, cc:cc + 1], css[:, cc:cc + 1])
                nc.vector.reciprocal_approx_fast(crn[:, cc:cc + 1],
                                                 csq[:, cc:cc + 1])

            for pr in range(4):
                prep_squares(pr, fast=True)
            bcs = [prep_finish(pr, defer_mult=True) for pr in range(4)]
            # k-outer so phase A can start after the first k-chunks land
            for k in range(KC):
                for pr in range(4):
                    nc.vector.tensor_mul(rhs8[pr][:, k, :], pT[pr][:, k, :],
                                         bcs[pr][:, :])

            def main_phase(half, preps=()):
                # 4 pairs; per (c, pair) one single-bank [128,512] PSUM tile
                # from a 6-deep ring; per-pair reduce_max drain (392 cols).
                preps = dict(preps)
                prs = list(range(half * 4, half * 4 + 4))
                for c in range(C):
                    pss = [psum.tile([128, 512], F32, tag="mm", bufs=6,
                                     name="mm") for _ in prs]
                    for j in range(NKP):
                        for i, pr in enumerate(prs):
                            nc.tensor.matmul(pss[i][:, 0:COLS],
                                             lhsT=cT[j][:, :, c * 128:(c + 1) * 128],
                                             rhs=rhs8[pr][:, 2 * j:2 * j + 2, :],
                                             start=(j == 0), stop=(j == NKP - 1),
                                             perf_mode=DR)
                    for i, pr in enumerate(prs):
                        nc.vector.reduce_max(
                            out=maxcol[:, c, 2 * pr:2 * pr + 2],
                            in_=pss[i][:, 0:VCOLS].rearrange(
                                "p (s x) -> p s x", s=2),
                            axis=AX.X)
                    for fn in preps.get(c, ()):
                        fn()

            # ---- IT-align: raw bf16 matmul + deferred rank-1 normalization --
            txtT = consts.tile([128, KC, 128], BF, tag="txtT")
            nc.sync.dma_start(out=txtT[:], in_=d_txtT.ap())
            imgT = consts.tile([128, KC, M_PER], BF, tag="imgT")
            nc.sync.dma_start(out=imgT[:], in_=d_imgT.ap())
            txtn = work.tile([128, D], BF, tag="cnat", bufs=3)
            nc.sync.dma_start(out=txtn[:], in_=d_txtn.ap())
            tscr = work.tile([128, D], BF, tag="scr", bufs=3)
            tss = small.tile([128, 1], F32, tag="tss")
            nc.scalar.activation(out=tscr[:], in_=txtn[:], func=AF.Square,
                                 accum_out=tss[:])
            nc.scalar.sqrt(tss[:], tss[:])
            av = small.tile([128, 1], F32, tag="av")
            nc.vector.reciprocal(av[:], tss[:])
            nc.vector.tensor_scalar_mul(av[:], av[:], float(t))  # t/||txt_v||

            # img norms via ones-matmul on squared imgT (transposed layout)
            isq = small.tile([128, KC, M_PER], BF, tag="isq")
            nc.scalar.activation(out=isq[:], in_=imgT[:], func=AF.Square)
            aux = psum.tile([128, 512], F32, tag="aux", bufs=2, name="aux")
            for k in range(KC):
                nc.tensor.matmul(aux[0:1, 0:M_PER], lhsT=ones_col[:, :],
                                 rhs=isq[:, k, :], start=(k == 0),
                                 stop=(k == KC - 1))
            ib = small.tile([1, M_PER], F32, tag="ib")
            nc.scalar.sqrt(ib[:], aux[0:1, 0:M_PER])
            nc.vector.reciprocal(ib[:], ib[:])                   # 1/||img_m||
            bg = small.tile([128, M_PER], F32, tag="bg")
            nc.gpsimd.partition_broadcast(out_ap=bg[:, :], in_ap=ib[0:1, :])

            itp = psum.tile([128, 512], F32, tag="aux", bufs=2, name="itp")
            for k in range(KC):
                nc.tensor.matmul(itp[:, 0:M_PER], lhsT=txtT[:, k, :],
                                 rhs=imgT[:, k, :], start=(k == 0),
                                 stop=(k == KC - 1))
            nc.scalar.activation(out=yit[:], in_=itp[:, 0:M_PER], func=AF.Copy,
                                 scale=av[:])
            nc.vector.tensor_mul(yit[:], yit[:], bg[:])
            nc.vector.tensor_scalar_add(yit[:], yit[:], float(bias))

            nc.scalar.activation(out=warm[:], in_=warm[:], func=AF.Exp)

            main_phase(0, preps={
                0: (lambda: prep_squares(4),),
                1: (lambda: prep_squares(5),),
                2: (lambda: prep_finish(4), lambda: prep_squares(6)),
                3: (lambda: prep_squares(7),),
                4: (lambda: prep_finish(5),),
                6: (lambda: prep_finish(6),),
                8: (lambda: prep_finish(7),),
            })

            # ---- S[v, m] = sum_p G[p,v] * maxcol[p,m]/||c_p||  ------------
            # (fp32 acc; accumulated chunk-by-chunk as phase B drains land)
            sps = psum.tile([128, 512], F32, tag="aux", bufs=2, name="sps")

            def s_step(cc):
                mcs = small.tile([128, M_PER], BF, tag="mcs", bufs=3)
                nc.vector.tensor_scalar_mul(mcs[:], maxcol[:, cc, :],
                                             crn[:, cc:cc + 1])
                nc.tensor.matmul(sps[:, 0:M_PER], lhsT=GTbf[:, cc * B:(cc + 1) * B],
                                 rhs=mcs[:], start=(cc == 0), stop=(cc == C - 1))

            def phase_b_prep(cc):
                fns = [lambda cc=cc: cnat_step(cc)]
                if cc >= 2:
                    fns.append(lambda cc=cc: s_step(cc - 2))
                return tuple(fns)

            main_phase(1, preps={cc: phase_b_prep(cc) for cc in range(C)})
            s_step(C - 2)
            s_step(C - 1)

            def softplus_out(y_ap, d_out):
                el = small.tile([B, M_PER], F32, tag="el", name="el")
                nc.scalar.activation(out=el[:], in_=y_ap, func=AF.Exp)
                nc.vector.tensor_scalar_add(el[:], el[:], 1.0)
                nc.scalar.activation(out=el[:], in_=el[:], func=AF.Ln)
                nc.sync.dma_start(out=d_out.ap(), in_=el[:])

            yrc = small.tile([B, M_PER], F32, tag="y")
            nc.scalar.activation(out=yrc[:], in_=sps[:, 0:M_PER], func=AF.Copy,
                                 bias=float(bias), scale=float(t))
            nc.vector.tensor_mul(yrc[:], yrc[:], sign[:])
            softplus_out(yrc[:], d_rc)

            nc.vector.tensor_mul(yit[:], yit[:], sign[:])
            softplus_out(yit[:], d_it)

    nc.compile()
    return nc


def _install_trace_hook():
    """Register the axon NTFF profiling hook (missing from this image) so
    run_bass_kernel_spmd(trace=True) can capture HW exec time."""
    import contextlib
    import ctypes
    import types

    import concourse.bass_utils as bu

    if "antenv.axon_hooks" in sys.modules:
        return
    so_path = "/opt/axon/libaxon_pjrt.so"

    def _make_hook():
        lib = ctypes.CDLL(so_path)
        if not hasattr(lib, "axon_start_nrt_profile"):
            return None
        lib.axon_start_nrt_profile.argtypes = [ctypes.POINTER(ctypes.c_int64),
                                               ctypes.c_size_t]
        lib.axon_start_nrt_profile.restype = ctypes.c_int64
        lib.axon_stop_nrt_profile.argtypes = [ctypes.c_char_p]
        lib.axon_stop_nrt_profile.restype = ctypes.c_int64

        @contextlib.contextmanager
        def _hook(output_dir, device_ids):
            import jax
            jax.devices()
            if device_ids:
                ids = (ctypes.c_int64 * len(device_ids))(*device_ids)
                rc = lib.axon_start_nrt_profile(ids, len(device_ids))
            else:
                rc = lib.axon_start_nrt_profile(None, 0)
            if rc != 0:
                raise RuntimeError(f"axon_start_nrt_profile rc={rc}")
            try:
                yield
            finally:
                n = lib.axon_stop_nrt_profile(str(output_dir).encode())
                print(f"profile: {n} file(s) written to {output_dir}",
                      file=sys.stderr)

        return _hook

    mod = types.ModuleType("antenv.axon_hooks")
    mod.get_axon_ntff_profile_hook = _make_hook
    sys.modules["antenv.axon_hooks"] = mod
    bu.upload_artifacts = lambda tmpdir: tmpdir  # no S3 in this container


def _prepare(inputs):
    image_features = np.asarray(inputs["image_features"], np.float32)
    text_features = np.asarray(inputs["text_features"], np.float32)
    image_token_features = np.asarray(inputs["image_token_features"], np.float32)
    concept_text_features = np.asarray(inputs["concept_text_features"], np.float32)
    counts = np.asarray(inputs["concept_counts"]).astype(np.int64)
    t = float(np.exp(np.clip(np.float32(inputs["logit_scale"]), -10.0, 10.0)))
    bias = float(np.float32(inputs["logit_bias"]))

    # pack concepts: keep only w < counts[v]; pad rows with ones (zero weight)
    vidx = np.repeat(np.arange(B), counts)
    widx = np.concatenate([np.arange(c) for c in counts])
    P = len(vidx)
    C = math.ceil(P / 128)
    P2 = C * 128
    cnat = np.ones((P2, D), np.float32)
    cnat[:P] = concept_text_features[vidx, widx]
    cnat_bf = cnat.astype(BF16)
    # cT[j, d128, h, p] = fp8(cnat[p, (2j+h)*128 + d])
    cT = np.ascontiguousarray(
        cnat.astype(FP8).T.reshape(NKP, 2, 128, P2).transpose(0, 2, 1, 3))

    # G with 1/(16*counts): folds away the x16 patch scale
    G = np.zeros((P2, B), np.float32)
    G[np.arange(P), vidx] = 1.0 / (16.0 * counts[vidx])
    GT = G.astype(BF16).reshape(C, 128, B)

    txt_bf = text_features.astype(BF16)
    # txtT[d, k, v] = txt_bf[v, k*128 + d]
    txtT = np.ascontiguousarray(
        txt_bf.T.reshape(KC, 128, B).transpose(1, 0, 2))

    in_maps = []
    for core in range(N_CORES):
        s = slice(core * M_PER, (core + 1) * M_PER)
        sh = image_token_features[s].astype(BF16)        # (16, 196, 768)
        pT = np.zeros((PAIRS, 128, KC, COLS), BF16)
        shT = sh.transpose(0, 2, 1).reshape(M_PER, KC, 128, NPATCH)
        # pT[pr, d, k, 196*i2 + n] = patches[2pr+i2][n, k*128+d]
        pT[:, :, :, 0:NPATCH] = shT[0::2].transpose(0, 2, 1, 3)
        pT[:, :, :, NPATCH:VCOLS] = shT[1::2].transpose(0, 2, 1, 3)
        img_bf = image_features[s].astype(BF16)          # (16, 768)
        imgT = np.ascontiguousarray(
            img_bf.T.reshape(KC, 128, M_PER).transpose(1, 0, 2))
        signneg = np.ones((B, M_PER), np.float32)
        for j in range(M_PER):
            signneg[core * M_PER + j, j] = -1.0
        in_maps.append({
            "pT": pT,
            "cT": cT,
            "cnat": cnat_bf,
            "GT": GT,
            "txtT": txtT,
            "imgT": imgT,
            "txtn": txt_bf,
            "signneg": signneg,
        })
    return in_maps, C, t, bias


def _run(inputs, trace=False, tmpdir=None):
    in_maps, C, t, bias = _prepare(inputs)
    key = (C, t, bias)
    if key not in _cache:
        _cache[key] = _build(C, t, bias)
    nc = _cache[key]
    kwargs = {}
    if trace:
        _install_trace_hook()
        kwargs = dict(trace=True, tmpdir=tmpdir)
    res = run_bass_kernel_spmd(nc, in_maps, core_ids=list(range(N_CORES)),
                               **kwargs)
    it_sum = sum(float(r["it_el"].astype(np.float64).sum()) for r in res.results)
    rc_sum = sum(float(r["rc_el"].astype(np.float64).sum()) for r in res.results)
    it_loss = it_sum / (B * B)
    rc_loss = rc_sum / (B * B)
    total = it_loss + 0.5 * rc_loss
    out = (np.float32(total), np.float32(it_loss), np.float32(rc_loss))
    return out, res


def kernel(**inputs):
    out, _ = _run(inputs)
    return out
